# revision 33
# baseline (speedup 1.0000x reference)
"""GateRow kernel for Trainium2 (8 NeuronCores, SPMD, gate-sharded, bit-packed).

Problem: out[b, g] = gates[g, 2*x[b, c0[g]] + x[b, c1[g]]]
  x: [16384, 8192] bool, gates: [8192, 4] bool, choices: [8192, 2] int32.

Strategy:
  Every 2-input boolean gate is  rowA OP rowB  for OP in {AND, OR, XOR}
  once operand inversion and constants are absorbed into a doubled
  lookup table TAB = [x^T ; ~x^T ; ones ; zeros] (one row per wire).
  Bit-pack the batch dimension (8 rows/byte) so each TAB row is
  B/8 = 2048 bytes and the boolean op is a plain bitwise op (done on
  uint32 views: bitwise is byte-local, and 32-bit elements quarter the
  DVE element count).

  Shard by GATES: core k owns 1024 gates.  Host sorts gates into
  type-homogeneous blocks of 128.  "Flexible" gates (constants and
  projections, f == one table row) are concentrated into pure-COPY
  blocks that skip the second gather and the ALU entirely: the
  gathered tile is DMAed straight to the output.  The remaining
  blocks are one stock tensor_tensor bitwise op each.  The schedule
  (#and/#or/#xor/#copy blocks per core) is derived from the actual
  gate-type counts at kernel() time and compiled per schedule.
  The host un-permutes output columns.

  Gathers use native indirect DMAs (InstDMACopy with a row offset per
  partition) — no gpsimd ucode library load (~9 us saved), descriptor
  generation is the Q7 firmware at ~8.7 ns/row.
"""

import sys

for _p in ("/opt/trn_rl_repo", "/opt/pypackages"):
    if _p not in sys.path:
        sys.path.append(_p)

from contextlib import ExitStack

import numpy as np

import concourse.bass as bass
import concourse.bacc as bacc
import concourse.tile as tile
import concourse.mybir as mybir
from concourse.bass_utils import run_bass_kernel_spmd

B, N, G, NCORES = 16384, 8192, 8192, 8
GPC = G // NCORES           # 1024 gates per core
NBLK = GPC // 128           # 8 gate blocks per core
PB = B // 8                 # 2048 packed bytes per table row
PW = PB // 4                # 512 packed uint32 words per table row
ROW_ONE = 2 * N             # all-ones table row
ROW_ZERO = 2 * N + 1        # all-zeros table row

# ---------------------------------------------------------------------------
# Gate classification.
#   tt bit (2a+b) = f(a, b).  Operand selectors:
#     0: x[c0]   1: ~x[c0]   2: x[c1]   3: ~x[c1]   4: ones   5: zeros
#   _SEL[op][tt] = (selA, selB) with f == rowA op rowB; None if
#   inexpressible.  _SEL["copy"][tt] = (selA, selA) when f == rowA.
# ---------------------------------------------------------------------------

_OPS = ("and", "or", "xor")
_NPOP = {"and": np.bitwise_and, "or": np.bitwise_or, "xor": np.bitwise_xor}


def _val(sel, a, b):
    return (a, 1 - a, b, 1 - b, 1, 0)[sel]


def _build_sel():
    sel = {op: [None] * 16 for op in (*_OPS, "copy")}
    for tt in range(16):
        for sa in range(6):
            if all(
                _val(sa, a, b) == ((tt >> (2 * a + b)) & 1)
                for a in (0, 1) for b in (0, 1)
            ):
                sel["copy"][tt] = (sa, sa)
                break
        for op in _OPS:
            for sa in range(6):
                for sb in range(6):
                    ok = all(
                        int(_NPOP[op](_val(sa, a, b), _val(sb, a, b)))
                        == ((tt >> (2 * a + b)) & 1)
                        for a in (0, 1) for b in (0, 1)
                    )
                    if ok and sel[op][tt] is None:
                        sel[op][tt] = (sa, sb)
    return sel


_SEL = _build_sel()
# Required family per tt: the single op that expresses it, or "copy".
_REQ = [
    "copy" if _SEL["copy"][tt] is not None
    else next(op for op in _OPS if _SEL[op][tt] is not None)
    for tt in range(16)
]


# ---------------------------------------------------------------------------
# Device program (parameterized by the per-core block schedule)
# ---------------------------------------------------------------------------

_ALU = {
    "and": mybir.AluOpType.bitwise_and,
    "or": mybir.AluOpType.bitwise_or,
    "xor": mybir.AluOpType.bitwise_xor,
}


def build_nc(sched, ncores=NCORES):
    """One SPMD program; all cores run it on their own gate shard.

    sched: tuple of NBLK block kinds ("and"/"or"/"xor"/"copy").
    Copy blocks gather one row per gate and DMA it straight out; op
    blocks gather two rows and run one tensor_tensor bitwise op.
    """
    ncalls = sum(2 if k in _OPS else 1 for k in sched)

    nc = bacc.Bacc(
        "TRN2",
        target_bir_lowering=False,
        debug=False,
        num_devices=ncores,
        num_swdge_queues=4,
    )
    tab = nc.dram_tensor("tab", [2 * N + 2, PW], mybir.dt.uint32, kind="ExternalInput")
    idxs = nc.dram_tensor("idxs", [128, ncalls], mybir.dt.int32, kind="ExternalInput")
    outd = nc.dram_tensor("out", [GPC, PW], mybir.dt.uint32, kind="ExternalOutput")

    with tile.TileContext(nc) as tc, ExitStack() as ctx:
        pconst = ctx.enter_context(tc.tile_pool(name="const", bufs=1))
        pg = ctx.enter_context(tc.tile_pool(name="gather", bufs=1))
        po = ctx.enter_context(tc.tile_pool(name="osb", bufs=2))

        idx_t = pconst.tile([128, ncalls], mybir.dt.int32)
        nc.sync.dma_start(idx_t[:], idxs[:])

        def gather(s, tag):
            g_t = pg.tile([128, PW], mybir.dt.uint32, tag=tag)
            bi = nc.gpsimd.indirect_dma_start(
                out=g_t[:],
                out_offset=None,
                in_=tab[:],
                in_offset=bass.IndirectOffsetOnAxis(ap=idx_t[:, s : s + 1], axis=0),
            )
            bi.ins.queue = f"qPoolDynamic{(s % 4) or ''}"
            return g_t

        s = 0
        for bk, kind in enumerate(sched):
            # Alternate the two HWDGE rings (sync=qSPDynamicHW,
            # scalar=qActDynamicHW) so output writes run in parallel.
            eng = nc.sync if bk % 2 == 0 else nc.scalar
            osl = outd[bk * 128 : (bk + 1) * 128, :]
            if kind == "copy":
                a_t = gather(s, f"g{s}")
                s += 1
                eng.dma_start(osl, a_t[:])
            else:
                a_t = gather(s, f"g{s}")
                b_t = gather(s + 1, f"g{s + 1}")
                s += 2
                o_t = po.tile([128, PW], mybir.dt.uint32, tag=f"o{bk}")
                nc.vector.tensor_tensor(o_t[:], a_t[:], b_t[:], op=_ALU[kind])
                eng.dma_start(osl, o_t[:])
        assert s == ncalls
    nc.compile()
    return nc


# ---------------------------------------------------------------------------
# Host-side input prep
# ---------------------------------------------------------------------------


def _prep(x, gates, choices):
    x8 = np.asarray(x, dtype=np.uint8)
    gates8 = np.asarray(gates, dtype=np.uint8)
    ch = np.asarray(choices, dtype=np.int64)

    # Packed doubled table (replicated on every core).
    xp = np.packbits(x8, axis=0)              # [B/8, N], bit MSB = lowest batch row
    tab = np.empty((2 * N + 2, PB), dtype=np.uint8)
    tab[:N] = xp.T
    tab[N : 2 * N] = ~tab[:N]
    tab[ROW_ONE] = 0xFF
    tab[ROW_ZERO] = 0x00
    tab32 = tab.view(np.uint32)

    # Data-driven schedule: block counts from the actual type census.
    tt = (gates8 << np.arange(4, dtype=np.uint8)).sum(axis=1).astype(np.int64)
    req = np.array([_REQ[t] for t in range(16)])[tt]    # per-gate family
    gid = np.arange(G)
    nblk = {op: -(-int((req == op).sum()) // (128 * NCORES)) for op in _OPS}
    bcopy = NBLK - sum(nblk.values())
    assert bcopy >= 0, f"schedule overflow: {nblk}"
    sched = sum(((op,) * nblk[op] for op in _OPS), ()) + ("copy",) * bcopy
    cap = {op: nblk[op] * 128 * NCORES for op in _OPS}
    cap["copy"] = bcopy * 128 * NCORES

    # Fill op buckets with their required gates, pad with copy-capable
    # gates; remaining copy gates fill the copy blocks exactly.
    flex_pool = gid[req == "copy"]
    fp = 0
    slots = {}
    for op in _OPS:
        need = gid[req == op]
        pad = cap[op] - len(need)
        assert pad >= 0
        slots[op] = np.concatenate([need, flex_pool[fp : fp + pad]])
        fp += pad
    slots["copy"] = flex_pool[fp:]
    assert len(slots["copy"]) == cap["copy"]

    # Device gate order (core-major, schedule-major) + operand rows.
    npc = {k: nblk.get(k, bcopy) * 128 for k in (*_OPS, "copy")}
    ncalls = sum(2 if k in _OPS else 1 for k in sched)
    perm = np.empty(G, dtype=np.int64)        # device row -> gate id
    offs = np.empty((NCORES, 128, ncalls), dtype=np.int32)
    r = 0
    for k in range(NCORES):
        s = 0
        for op in (*_OPS, "copy"):
            g = slots[op][k * npc[op] : (k + 1) * npc[op]]
            lut = [_SEL[op][t] or (5, 5) for t in range(16)]
            selA = np.array([q[0] for q in lut])[tt[g]]
            selB = np.array([q[1] for q in lut])[tt[g]]
            rows = np.stack(
                [ch[g, 0], ch[g, 0] + N, ch[g, 1], ch[g, 1] + N,
                 np.full(len(g), ROW_ONE), np.full(len(g), ROW_ZERO)]
            )
            n = len(g)
            perm[r : r + n] = g
            ra = rows[selA, np.arange(n)].reshape(-1, 128)
            rb = rows[selB, np.arange(n)].reshape(-1, 128)
            for j in range(n // 128):
                offs[k, :, s] = ra[j]
                s += 1
                if op != "copy":
                    offs[k, :, s] = rb[j]
                    s += 1
            r += n
        assert s == ncalls
    assert r == G

    in_maps = [
        {"tab": tab32, "idxs": np.ascontiguousarray(offs[k])} for k in range(NCORES)
    ]
    return in_maps, perm, sched


# ---------------------------------------------------------------------------
# Entry point
# ---------------------------------------------------------------------------

_NC_CACHE = {}


def _get_nc(sched):
    if sched not in _NC_CACHE:
        _NC_CACHE[sched] = build_nc(sched)
    return _NC_CACHE[sched]


def kernel(x, gates, choices):
    in_maps, perm, sched = _prep(x, gates, choices)
    nc = _get_nc(sched)
    res = run_bass_kernel_spmd(nc, in_maps, list(range(NCORES)))
    packed = np.concatenate(
        [res.results[k]["out"].view(np.uint8) for k in range(NCORES)], axis=0
    )
    ordered = np.empty_like(packed)
    ordered[perm] = packed                    # un-permute gate rows
    up = np.unpackbits(ordered, axis=1)       # [G, B] 0/1 uint8
    return up.view(np.bool_).T                # [B, G] bool view
